# revision 29
# baseline (speedup 1.0000x reference)
"""DistSageConv on 8 TRN2 NeuronCores (Bass/Tile) — aggregate-first, no gather.

Reference computation:
    out  = x @ W1.T + b1                                  # [n_src, 128]
    out1 = segment_sum(out[src_ids], dst_ids, n_dst)      # [n_dst, 128]
    out5 = x[:n_dst] @ W2.T + b2
    return out5 + out1

Key identity: segment_sum(xg @ W1T) = segment_sum(xg) @ W1T — aggregate the
raw 256-dim x rows per dst first (cheap one-hot matmuls with K=edges), then
project each 128-row dst block once. The b1 term is deg ⊗ b1, folded into the
out5 matmul via an augmented K=258 operand (rows 256=deg, 257=ones).

Distribution: dst blocks sharded across 8 cores (40 blocks of 128 each);
edges arrive host-sorted by dst, so each core's edges are a contiguous run.
The host ships per-edge x rows (pure index marshaling) laid out per
(block, tile) with every block padded to a fixed TPB tiles of 128 edges —
a single static SPMD program, no data-dependent control, no inter-core
communication, no SWDGE gather (the 1ms GpSimd descriptor-gen bottleneck of
the gather-based design).

Per core device program:
  phase A (early): out5 = xdT-augmented @ W2Ta into OUT5 (f32 SBUF)
  phase B: for each block b, tile t: one-hot O[e,d] = (dstloc==iota);
           psum_aggT[xfeat, d] += xg[:,half].T @ O  (2 matmuls, K=128 edges)
       per block: aggT -> bf16; S_b[d,f] = aggT.T @ W1T (2 matmuls, K=xfeat);
           out rows = S_b + OUT5[:, b] -> DRAM
"""
import sys
sys.path.insert(0, "/opt/trn_rl_repo")

import numpy as np
import ml_dtypes

import os
import concourse.bacc as bacc
import concourse.bass as bass
import concourse.mybir as mybir
import concourse.tile as tile
from concourse.bass_utils import run_bass_kernel_spmd

# ---------------- problem constants (hardcoded per contract) --------------
P = 8                      # cores
N_SRC = 100000
N_DST = 40000
N_EDGES = 640000
INF = 256                  # in_feats
OUTF = 128                 # out_feats
NBLK = 320                 # padded dst blocks of 128 (40960 dst rows)
BPC = NBLK // P            # 40 blocks per core
DPC = BPC * 128            # 5120 dst rows per core

F32 = mybir.dt.float32
BF16 = mybir.dt.bfloat16

_CACHE = {}


# ============================ host-side prep ==============================

def _host_prep(x, W1, b1, W2, b2, src_ids, dst_ids):
    x = np.asarray(x, np.float32)
    W1 = np.asarray(W1, np.float32)
    W2 = np.asarray(W2, np.float32)
    b1 = np.asarray(b1, np.float32).reshape(-1)
    b2 = np.asarray(b2, np.float32).reshape(-1)
    src_ids = np.asarray(src_ids, np.int64)
    dst_ids = np.asarray(dst_ids, np.int64)

    order = np.argsort(dst_ids, kind="stable")
    src_s = src_ids[order]
    dst_s = dst_ids[order]

    deg = np.bincount(dst_s, minlength=NBLK * 128).astype(np.float32)
    cb = np.bincount(dst_s // 128, minlength=NBLK)        # edges per block
    tiles_g = np.maximum(1, -(-cb // 128))                # tiles per block
    # balanced slot assignment: sort blocks by tile count desc, groups of 8;
    # slot j holds sorted blocks [8j, 8j+8) (one per core) -> per-slot max
    # (= shared schedule) is minimal, and small blocks land at the tail
    border = np.argsort(-tiles_g, kind="stable")          # [NBLK]
    assign = border.reshape(BPC, P).T                     # [P, BPC] global blk
    core_of_blk = np.zeros(NBLK, dtype=np.int64)
    slot_of_blk = np.zeros(NBLK, dtype=np.int64)
    for c in range(P):
        core_of_blk[assign[c]] = c
        slot_of_blk[assign[c]] = np.arange(BPC)
    tpb = tiles_g[assign[0]]                              # [BPC] slot maxima
    toff = np.zeros(BPC + 1, dtype=np.int64)
    toff[1:] = np.cumsum(tpb)
    NT = int(toff[-1])                                    # tiles per core

    # slot position of every edge: per-core [NT*128] slot arrays
    bstart = np.zeros(NBLK + 1, dtype=np.int64)
    bstart[1:] = np.cumsum(cb)
    blkg = dst_s // 128
    pos_in_blk = np.arange(len(dst_s)) - bstart[blkg]
    gpos = (core_of_blk[blkg] * NT + toff[slot_of_blk[blkg]]) * 128 + pos_in_blk

    slot_src = np.zeros(P * NT * 128, dtype=np.int64)
    dstloc = np.full(P * NT * 128, -1.0, dtype=np.float32)
    slot_src[gpos] = src_s
    dstloc[gpos] = (dst_s - (dst_s // 128) * 128).astype(np.float32)

    xg_all = x[slot_src].astype(ml_dtypes.bfloat16)       # [P*NT*128, 256]
    # partition-major per core: [128 (edge-in-tile), NT, 256]
    xg_pm = xg_all.reshape(P, NT, 128, INF).transpose(0, 2, 1, 3)
    # dstloc layout per core: [128 (edge-in-tile), NT]
    dl = dstloc.reshape(P, NT, 128).transpose(0, 2, 1)
    dl = np.ascontiguousarray(dl.astype(ml_dtypes.bfloat16))

    TPBMAX = int(tpb.max())
    iota3 = np.broadcast_to(np.arange(128, dtype=np.float32),
                            (128, TPBMAX, 128)).reshape(128, TPBMAX * 128)
    iota3 = np.ascontiguousarray(iota3.astype(ml_dtypes.bfloat16))
    ident = np.ascontiguousarray(np.eye(128, dtype=np.float32)
                                 .astype(ml_dtypes.bfloat16))
    W1T = np.ascontiguousarray(W1.T.astype(ml_dtypes.bfloat16))   # [256, 128]
    W2T_aug = np.concatenate([W2.T, b1[None, :], b2[None, :]], axis=0)
    W2T_aug = np.ascontiguousarray(W2T_aug.astype(ml_dtypes.bfloat16))

    xpad = np.zeros((NBLK * 128, INF), dtype=np.float32)
    xpad[:N_DST] = x[:N_DST]
    in_maps = []
    for c in range(P):
        didx = (assign[c][:, None] * 128 + np.arange(128)[None, :]).reshape(-1)
        xdT = np.zeros((INF + 2, DPC), dtype=np.float32)
        xdT[:INF, :] = xpad[didx].T
        xdT[INF, :] = deg[didx]
        xdT[INF + 1, :] = 1.0
        in_maps.append({
            "xg": np.ascontiguousarray(xg_pm[c]),
            "dstloc": dl[c],
            "xdT": np.ascontiguousarray(xdT.astype(ml_dtypes.bfloat16)),
            "W1T": W1T,
            "W2Ta": W2T_aug,
            "iota3": iota3,
            "ident": ident,
        })
    return in_maps, tuple(int(t) for t in tpb), assign


# ============================ device program ==============================

def _build(tpb):
    NT = int(sum(tpb))
    toff = [0]
    for t in tpb:
        toff.append(toff[-1] + t)

    nc = bacc.Bacc("TRN2", target_bir_lowering=False, debug=False,
                   num_devices=P)

    xg_d = nc.dram_tensor("xg", [128, NT, INF], BF16, kind="ExternalInput")
    dl_d = nc.dram_tensor("dstloc", [128, NT], BF16, kind="ExternalInput")
    xdT_d = nc.dram_tensor("xdT", [INF + 2, DPC], BF16, kind="ExternalInput")
    W1T_d = nc.dram_tensor("W1T", [INF, OUTF], BF16, kind="ExternalInput")
    W2Ta_d = nc.dram_tensor("W2Ta", [INF + 2, OUTF], BF16,
                            kind="ExternalInput")
    TPBMAX = int(max(tpb))
    iota3_d = nc.dram_tensor("iota3", [128, TPBMAX * 128], BF16,
                             kind="ExternalInput")
    ident_d = nc.dram_tensor("ident", [128, 128], BF16, kind="ExternalInput")
    out_d = nc.dram_tensor("out", [DPC, OUTF], BF16, kind="ExternalOutput")

    OUT5 = nc.alloc_sbuf_tensor("out5", [128, BPC * 128], BF16)

    with tile.TileContext(nc) as tc:
        with (
            tc.tile_pool(name="consts", bufs=1) as constp,
            tc.tile_pool(name="xg", bufs=8) as xgp,
            tc.tile_pool(name="oh", bufs=4) as ohp,
            tc.tile_pool(name="agc", bufs=3) as agcp,
            tc.tile_pool(name="ost", bufs=3) as ostp,
            tc.tile_pool(name="psa0", bufs=2, space="PSUM") as psa0p,
            tc.tile_pool(name="psa1", bufs=2, space="PSUM") as psa1p,
            tc.tile_pool(name="pss", bufs=2, space="PSUM") as pssp,
            tc.tile_pool(name="ps3", bufs=2, space="PSUM") as ps3p,
        ):
            # ---- constants
            iota3_t = constp.tile([128, TPBMAX * 128], BF16)
            nc.sync.dma_start(iota3_t[:], iota3_d[:])
            ident_t = constp.tile([128, 128], BF16)
            nc.sync.dma_start(ident_t[:], ident_d[:])
            w1 = constp.tile([128, 2, OUTF], BF16)
            nc.sync.dma_start(w1[:], W1T_d[:].rearrange("(k p) f -> p k f", p=128))
            w2 = constp.tile([128, 2, OUTF], BF16)
            nc.sync.dma_start(w2[:], W2Ta_d[:INF].rearrange("(k p) f -> p k f", p=128))
            wb = constp.tile([2, OUTF], BF16)
            nc.sync.dma_start(wb[:], W2Ta_d[INF:INF + 2, :])

            # ---- whole-tensor loads: dstloc + xdT (3 DMAs instead of 70)
            dl_t = constp.tile([128, NT], BF16)
            nc.scalar.dma_start(dl_t[:], dl_d[:])
            b0a = constp.tile([128, DPC], BF16)
            nc.scalar.dma_start(b0a[:], xdT_d[0:128, :])
            b1a = constp.tile([128, DPC], BF16)
            nc.scalar.dma_start(b1a[:], xdT_d[128:256, :])
            b2a = constp.tile([2, DPC], BF16)
            nc.scalar.dma_start(b2a[:], xdT_d[256:258, :])

            def emit_phase_a_group(grp):
                ps3 = ps3p.tile([128, 512], F32, space="PSUM", tag="p3")
                for u in range(4):
                    sl = slice(grp * 512 + u * 128, grp * 512 + (u + 1) * 128)
                    osl = slice(u * 128, (u + 1) * 128)
                    nc.tensor.matmul(out=ps3[:, osl], lhsT=b0a[:, sl],
                                     rhs=w2[:, 0, :], start=(u == 0), stop=False)
                    nc.tensor.matmul(out=ps3[:, osl], lhsT=b1a[:, sl],
                                     rhs=w2[:, 1, :], start=False, stop=False)
                    nc.tensor.matmul(out=ps3[:, osl], lhsT=b2a[:, sl],
                                     rhs=wb[:], start=False, stop=(u == 3))
                nc.scalar.copy(
                    out=OUT5[:, grp * 512:(grp + 1) * 512], in_=ps3[:])

            # -------- phase B (phase A groups interleaved every 4 blocks) ---
            with nc.named_scope("phaseB"):
                for b in range(BPC):
                    if b % 4 == 0:
                        emit_phase_a_group(b // 4)
                    TPB = tpb[b]
                    xgb = xgp.tile([128, TPB, INF], BF16, tag="xg")
                    nc.sync.dma_start(xgb[:], xg_d[:, toff[b]:toff[b + 1], :])
                    off = 0
                    oh3 = ohp.tile([128, TPB, 128], BF16, tag="oh")
                    nc.vector.tensor_tensor(
                        out=oh3[:],
                        in0=iota3_t[:, :TPB * 128]
                            .rearrange("p (t f) -> p t f", t=TPB),
                        in1=dl_t[:, toff[b]:toff[b + 1]]
                            .to_broadcast([128, TPB, 128]),
                        op=mybir.AluOpType.is_equal)
                    psa0 = psa0p.tile([128, 128], F32, space="PSUM", tag="a0")
                    psa1 = psa1p.tile([128, 128], F32, space="PSUM", tag="a1")
                    for t in range(TPB):
                        nc.tensor.matmul(
                            out=psa0[:],
                            lhsT=xgb[:, off + t, 0:128],
                            rhs=oh3[:, t, :],
                            start=(t == 0), stop=(t == TPB - 1))
                        nc.tensor.matmul(
                            out=psa1[:],
                            lhsT=xgb[:, off + t, 128:256],
                            rhs=oh3[:, t, :],
                            start=(t == 0), stop=(t == TPB - 1))
                    agc = agcp.tile([128, 2, 128], BF16, tag="ag")
                    nc.scalar.copy(out=agc[:, 0, :], in_=psa0[:])
                    nc.scalar.copy(out=agc[:, 1, :], in_=psa1[:])
                    pss = pssp.tile([128, 128], F32, space="PSUM", tag="s")
                    nc.tensor.matmul(out=pss[:], lhsT=agc[:, 0, :],
                                     rhs=w1[:, 0, :], start=True, stop=False)
                    nc.tensor.matmul(out=pss[:], lhsT=agc[:, 1, :],
                                     rhs=w1[:, 1, :], start=False, stop=False)
                    nc.tensor.matmul(out=pss[:], lhsT=ident_t[:],
                                     rhs=OUT5[:, b * 128:(b + 1) * 128],
                                     start=False, stop=True)
                    ost = ostp.tile([128, 128], BF16, tag="o")
                    nc.scalar.copy(out=ost[:], in_=pss[:])
                    nc.scalar.dma_start(out_d[b * 128:(b + 1) * 128, :], ost[:])
    nc.compile()
    return nc


# ============================ public entry ================================

def _install_ntff_hook():
    """The agent image lacks antenv.axon_hooks; recreate it and register the
    ctypes NTFF profile hook so trace=True works under axon."""
    import types
    import antenv
    if "antenv.axon_hooks" not in sys.modules:
        m = types.ModuleType("antenv.axon_hooks")
        _h = [None]
        m.get_axon_ntff_profile_hook = lambda: _h[0]
        m.set_axon_ntff_profile_hook = lambda h: _h.__setitem__(0, h)
        sys.modules["antenv.axon_hooks"] = m
        antenv.axon_hooks = m
    import antenv.axon_hooks as ah
    if ah.get_axon_ntff_profile_hook() is None:
        try:
            from trn_agent_boot.trn_boot import _ntff_profile_via_ctypes
            ah.set_axon_ntff_profile_hook(
                _ntff_profile_via_ctypes("/opt/axon/libaxon_pjrt.so"))
        except Exception as e:
            print(f"ntff hook install failed ({e}); timing disabled")


def kernel(x, W1, b1, W2, b2, src_ids, dst_ids, n_dst):
    n_dst = int(n_dst)
    assert n_dst == N_DST
    in_maps, tpb, assign = _host_prep(x, W1, b1, W2, b2, src_ids, dst_ids)
    if tpb not in _CACHE:
        _CACHE.clear()
        _CACHE[tpb] = _build(tpb)
    nc = _CACHE[tpb]
    trace = bool(os.environ.get("BASS_KERNEL_TRACE"))
    kw = {}
    if trace:
        _install_ntff_hook()
        kw = dict(trace=True, trace_cores=[0], stitch_traces=False)
    res = run_bass_kernel_spmd(nc, in_maps, core_ids=list(range(P)), **kw)
    if trace:
        print(f"HW exec time: {res.exec_time_ns} ns")
        if res.per_core_scope_times:
            for scope, m in sorted(res.per_core_scope_times.items()):
                print(f"  scope {scope}: {m}")
        if res.instructions_and_trace:
            print(f"  trace: {res.instructions_and_trace[1]}")
    out = np.zeros((NBLK * 128, OUTF), dtype=np.float32)
    for c in range(P):
        didx = (assign[c][:, None] * 128 + np.arange(128)[None, :]).reshape(-1)
        out[didx] = np.asarray(res.results[c]["out"], np.float32)
    return np.ascontiguousarray(out[:N_DST])


if __name__ == "__main__":
    # smoke test with random data
    rng = np.random.default_rng(0)
    x = rng.standard_normal((N_SRC, INF), dtype=np.float32)
    W1 = rng.standard_normal((OUTF, INF), dtype=np.float32) * 0.0625
    W2 = rng.standard_normal((OUTF, INF), dtype=np.float32) * 0.0625
    b1 = np.zeros(OUTF, np.float32)
    b2 = np.zeros(OUTF, np.float32)
    src = rng.integers(0, N_SRC, N_EDGES).astype(np.int32)
    dst = np.sort(rng.integers(0, N_DST, N_EDGES).astype(np.int32))
    got = kernel(x, W1, b1, W2, b2, src, dst, N_DST)
    proj = x @ W1.T + b1
    want = np.zeros((N_DST, OUTF), np.float32)
    np.add.at(want, dst, proj[src])
    want += x[:N_DST] @ W2.T + b2
    denom = np.abs(want).max()
    print("rel err:", np.abs(got - want).max() / denom)


# revision 31
# speedup vs baseline: 1.0806x; 1.0806x over previous
"""DistSageConv on 8 TRN2 NeuronCores (Bass/Tile) — aggregate-first, no gather.

Reference computation:
    out  = x @ W1.T + b1                                  # [n_src, 128]
    out1 = segment_sum(out[src_ids], dst_ids, n_dst)      # [n_dst, 128]
    out5 = x[:n_dst] @ W2.T + b2
    return out5 + out1

Key identity: segment_sum(xg @ W1T) = segment_sum(xg) @ W1T — aggregate the
raw 256-dim x rows per dst first (cheap one-hot matmuls with K=edges), then
project each 128-row dst block once. The b1 term is deg ⊗ b1, folded into the
out5 matmul via an augmented K=258 operand (rows 256=deg, 257=ones).

Distribution: dst blocks sharded across 8 cores (40 blocks of 128 each);
edges arrive host-sorted by dst, so each core's edges are a contiguous run.
The host ships per-edge x rows (pure index marshaling) laid out per
(block, tile) with every block padded to a fixed TPB tiles of 128 edges —
a single static SPMD program, no data-dependent control, no inter-core
communication, no SWDGE gather (the 1ms GpSimd descriptor-gen bottleneck of
the gather-based design).

Per core device program:
  phase A (early): out5 = xdT-augmented @ W2Ta into OUT5 (f32 SBUF)
  phase B: for each block b, tile t: one-hot O[e,d] = (dstloc==iota);
           psum_aggT[xfeat, d] += xg[:,half].T @ O  (2 matmuls, K=128 edges)
       per block: aggT -> bf16; S_b[d,f] = aggT.T @ W1T (2 matmuls, K=xfeat);
           out rows = S_b + OUT5[:, b] -> DRAM
"""
import sys
sys.path.insert(0, "/opt/trn_rl_repo")

import numpy as np
import ml_dtypes

import os
import concourse.bacc as bacc
import concourse.bass as bass
import concourse.mybir as mybir
import concourse.tile as tile
from concourse.bass_utils import run_bass_kernel_spmd

# ---------------- problem constants (hardcoded per contract) --------------
P = 8                      # cores
N_SRC = 100000
N_DST = 40000
N_EDGES = 640000
INF = 256                  # in_feats
OUTF = 128                 # out_feats
NBLK = 320                 # padded dst blocks of 128 (40960 dst rows)
BPC = NBLK // P            # 40 blocks per core
DPC = BPC * 128            # 5120 dst rows per core

F32 = mybir.dt.float32
BF16 = mybir.dt.bfloat16

_CACHE = {}


# ============================ host-side prep ==============================

def _host_prep(x, W1, b1, W2, b2, src_ids, dst_ids):
    x = np.asarray(x, np.float32)
    W1 = np.asarray(W1, np.float32)
    W2 = np.asarray(W2, np.float32)
    b1 = np.asarray(b1, np.float32).reshape(-1)
    b2 = np.asarray(b2, np.float32).reshape(-1)
    src_ids = np.asarray(src_ids, np.int64)
    dst_ids = np.asarray(dst_ids, np.int64)

    order = np.argsort(dst_ids, kind="stable")
    src_s = src_ids[order]
    dst_s = dst_ids[order]

    deg = np.bincount(dst_s, minlength=NBLK * 128).astype(np.float32)
    cb = np.bincount(dst_s // 128, minlength=NBLK)        # edges per block
    tiles_g = np.maximum(1, -(-cb // 128))                # tiles per block
    # balanced slot assignment: sort blocks by tile count desc, groups of 8;
    # slot j holds sorted blocks [8j, 8j+8) (one per core) -> per-slot max
    # (= shared schedule) is minimal, and small blocks land at the tail
    border = np.argsort(-tiles_g, kind="stable")          # [NBLK]
    assign = border.reshape(BPC, P).T                     # [P, BPC] global blk
    core_of_blk = np.zeros(NBLK, dtype=np.int64)
    slot_of_blk = np.zeros(NBLK, dtype=np.int64)
    for c in range(P):
        core_of_blk[assign[c]] = c
        slot_of_blk[assign[c]] = np.arange(BPC)
    tpb = tiles_g[assign[0]]                              # [BPC] slot maxima
    toff = np.zeros(BPC + 1, dtype=np.int64)
    toff[1:] = np.cumsum(tpb)
    NT = int(toff[-1])                                    # tiles per core

    # slot position of every edge: per-core [NT*128] slot arrays
    bstart = np.zeros(NBLK + 1, dtype=np.int64)
    bstart[1:] = np.cumsum(cb)
    blkg = dst_s // 128
    pos_in_blk = np.arange(len(dst_s)) - bstart[blkg]
    gpos = (core_of_blk[blkg] * NT + toff[slot_of_blk[blkg]]) * 128 + pos_in_blk

    slot_src = np.zeros(P * NT * 128, dtype=np.int64)
    dstloc = np.full(P * NT * 128, -1.0, dtype=np.float32)
    slot_src[gpos] = src_s
    dstloc[gpos] = (dst_s - (dst_s // 128) * 128).astype(np.float32)

    xg_all = x[slot_src].astype(ml_dtypes.bfloat16)       # [P*NT*128, 256]
    # partition-major per core: [128 (edge-in-tile), NT, 256]
    xg_pm = xg_all.reshape(P, NT, 128, INF).transpose(0, 2, 1, 3)
    # dstloc layout per core: [128 (edge-in-tile), NT]
    dl = dstloc.reshape(P, NT, 128).transpose(0, 2, 1)
    dl = np.ascontiguousarray(dl.astype(ml_dtypes.bfloat16))

    TPBMAX = int(tpb.max())
    iota3 = np.broadcast_to(np.arange(128, dtype=np.float32),
                            (128, TPBMAX, 128)).reshape(128, TPBMAX * 128)
    iota3 = np.ascontiguousarray(iota3.astype(ml_dtypes.bfloat16))
    ident = np.ascontiguousarray(np.eye(128, dtype=np.float32)
                                 .astype(ml_dtypes.bfloat16))
    W1T = np.ascontiguousarray(W1.T.astype(ml_dtypes.bfloat16))   # [256, 128]
    W2T_aug = np.concatenate([W2.T, b1[None, :], b2[None, :]], axis=0)
    W2T_aug = np.ascontiguousarray(W2T_aug.astype(ml_dtypes.bfloat16))

    xpad = np.zeros((NBLK * 128, INF), dtype=np.float32)
    xpad[:N_DST] = x[:N_DST]
    in_maps = []
    for c in range(P):
        didx = (assign[c][:, None] * 128 + np.arange(128)[None, :]).reshape(-1)
        xdT = np.zeros((INF + 2, DPC), dtype=np.float32)
        xdT[:INF, :] = xpad[didx].T
        xdT[INF, :] = deg[didx]
        xdT[INF + 1, :] = 1.0
        in_maps.append({
            "xg": np.ascontiguousarray(xg_pm[c]),
            "dstloc": dl[c],
            "xdT": np.ascontiguousarray(xdT.astype(ml_dtypes.bfloat16)),
            "W1T": W1T,
            "W2Ta": W2T_aug,
            "iota3": iota3,
            "ident": ident,
        })
    return in_maps, tuple(int(t) for t in tpb), assign


# ============================ device program ==============================

def _build(tpb):
    NT = int(sum(tpb))
    toff = [0]
    for t in tpb:
        toff.append(toff[-1] + t)

    nc = bacc.Bacc("TRN2", target_bir_lowering=False, debug=False,
                   num_devices=P)

    xg_d = nc.dram_tensor("xg", [128, NT, INF], BF16, kind="ExternalInput")
    dl_d = nc.dram_tensor("dstloc", [128, NT], BF16, kind="ExternalInput")
    xdT_d = nc.dram_tensor("xdT", [INF + 2, DPC], BF16, kind="ExternalInput")
    W1T_d = nc.dram_tensor("W1T", [INF, OUTF], BF16, kind="ExternalInput")
    W2Ta_d = nc.dram_tensor("W2Ta", [INF + 2, OUTF], BF16,
                            kind="ExternalInput")
    TPBMAX = int(max(tpb))
    iota3_d = nc.dram_tensor("iota3", [128, TPBMAX * 128], BF16,
                             kind="ExternalInput")
    ident_d = nc.dram_tensor("ident", [128, 128], BF16, kind="ExternalInput")
    out_d = nc.dram_tensor("out", [DPC, OUTF], BF16, kind="ExternalOutput")

    OUT5 = nc.alloc_sbuf_tensor("out5", [128, BPC * 128], BF16)

    with tile.TileContext(nc) as tc:
        with (
            tc.tile_pool(name="consts", bufs=1) as constp,
            tc.tile_pool(name="xg", bufs=10) as xgp,
            tc.tile_pool(name="oh", bufs=6) as ohp,
            tc.tile_pool(name="agc", bufs=3) as agcp,
            tc.tile_pool(name="ost", bufs=3) as ostp,
            tc.tile_pool(name="psa0", bufs=2, space="PSUM") as psa0p,
            tc.tile_pool(name="psa1", bufs=2, space="PSUM") as psa1p,
            tc.tile_pool(name="pss", bufs=2, space="PSUM") as pssp,
            tc.tile_pool(name="ps3", bufs=2, space="PSUM") as ps3p,
        ):
            # ---- constants
            iota3_t = constp.tile([128, TPBMAX * 128], BF16)
            nc.scalar.dma_start(iota3_t[:], iota3_d[:])
            ident_t = constp.tile([128, 128], BF16)
            nc.scalar.dma_start(ident_t[:], ident_d[:])
            w1 = constp.tile([128, 2, OUTF], BF16)
            nc.sync.dma_start(w1[:], W1T_d[:].rearrange("(k p) f -> p k f", p=128))
            w2 = constp.tile([128, 2, OUTF], BF16)
            nc.sync.dma_start(w2[:], W2Ta_d[:INF].rearrange("(k p) f -> p k f", p=128))
            wb = constp.tile([2, OUTF], BF16)
            nc.sync.dma_start(wb[:], W2Ta_d[INF:INF + 2, :])

            # ---- whole-tensor loads: dstloc + xdT (3 DMAs instead of 70)
            dl_t = constp.tile([128, NT], BF16)
            nc.scalar.dma_start(dl_t[:], dl_d[:])
            b0a = constp.tile([128, DPC], BF16)
            nc.scalar.dma_start(b0a[:], xdT_d[0:128, :])
            b1a = constp.tile([128, DPC], BF16)
            nc.scalar.dma_start(b1a[:], xdT_d[128:256, :])
            b2a = constp.tile([2, DPC], BF16)
            nc.scalar.dma_start(b2a[:], xdT_d[256:258, :])

            def emit_phase_a_group(grp):
                ps3 = ps3p.tile([128, 512], F32, space="PSUM", tag="p3")
                for u in range(4):
                    sl = slice(grp * 512 + u * 128, grp * 512 + (u + 1) * 128)
                    osl = slice(u * 128, (u + 1) * 128)
                    nc.tensor.matmul(out=ps3[:, osl], lhsT=b0a[:, sl],
                                     rhs=w2[:, 0, :], start=(u == 0), stop=False)
                    nc.tensor.matmul(out=ps3[:, osl], lhsT=b1a[:, sl],
                                     rhs=w2[:, 1, :], start=False, stop=False)
                    nc.tensor.matmul(out=ps3[:, osl], lhsT=b2a[:, sl],
                                     rhs=wb[:], start=False, stop=(u == 3))
                nc.scalar.copy(
                    out=OUT5[:, grp * 512:(grp + 1) * 512], in_=ps3[:])

            # -------- phase B (phase A groups interleaved every 4 blocks) ---
            with nc.named_scope("phaseB"):
                for b in range(BPC):
                    if b % 4 == 0:
                        emit_phase_a_group(b // 4)
                    TPB = tpb[b]
                    xgb = xgp.tile([128, TPB, INF], BF16, tag="xg")
                    nc.sync.dma_start(xgb[:], xg_d[:, toff[b]:toff[b + 1], :])
                    off = 0
                    oh3 = ohp.tile([128, TPB, 128], BF16, tag="oh")
                    nc.vector.tensor_tensor(
                        out=oh3[:],
                        in0=iota3_t[:, :TPB * 128]
                            .rearrange("p (t f) -> p t f", t=TPB),
                        in1=dl_t[:, toff[b]:toff[b + 1]]
                            .to_broadcast([128, TPB, 128]),
                        op=mybir.AluOpType.is_equal)
                    psa0 = psa0p.tile([128, 128], F32, space="PSUM", tag="a0")
                    psa1 = psa1p.tile([128, 128], F32, space="PSUM", tag="a1")
                    for t in range(TPB):
                        nc.tensor.matmul(
                            out=psa0[:],
                            lhsT=xgb[:, off + t, 0:128],
                            rhs=oh3[:, t, :],
                            start=(t == 0), stop=(t == TPB - 1))
                        nc.tensor.matmul(
                            out=psa1[:],
                            lhsT=xgb[:, off + t, 128:256],
                            rhs=oh3[:, t, :],
                            start=(t == 0), stop=(t == TPB - 1))
                    agc = agcp.tile([128, 2, 128], BF16, tag="ag")
                    nc.scalar.copy(out=agc[:, 0, :], in_=psa0[:])
                    nc.scalar.copy(out=agc[:, 1, :], in_=psa1[:])
                    pss = pssp.tile([128, 128], F32, space="PSUM", tag="s")
                    nc.tensor.matmul(out=pss[:], lhsT=agc[:, 0, :],
                                     rhs=w1[:, 0, :], start=True, stop=False)
                    nc.tensor.matmul(out=pss[:], lhsT=agc[:, 1, :],
                                     rhs=w1[:, 1, :], start=False, stop=False)
                    nc.tensor.matmul(out=pss[:], lhsT=ident_t[:],
                                     rhs=OUT5[:, b * 128:(b + 1) * 128],
                                     start=False, stop=True)
                    ost = ostp.tile([128, 128], BF16, tag="o")
                    nc.scalar.copy(out=ost[:], in_=pss[:])
                    nc.scalar.dma_start(out_d[b * 128:(b + 1) * 128, :], ost[:])
    nc.compile()
    return nc


# ============================ public entry ================================

def _install_ntff_hook():
    """The agent image lacks antenv.axon_hooks; recreate it and register the
    ctypes NTFF profile hook so trace=True works under axon."""
    import types
    import antenv
    if "antenv.axon_hooks" not in sys.modules:
        m = types.ModuleType("antenv.axon_hooks")
        _h = [None]
        m.get_axon_ntff_profile_hook = lambda: _h[0]
        m.set_axon_ntff_profile_hook = lambda h: _h.__setitem__(0, h)
        sys.modules["antenv.axon_hooks"] = m
        antenv.axon_hooks = m
    import antenv.axon_hooks as ah
    if ah.get_axon_ntff_profile_hook() is None:
        try:
            from trn_agent_boot.trn_boot import _ntff_profile_via_ctypes
            ah.set_axon_ntff_profile_hook(
                _ntff_profile_via_ctypes("/opt/axon/libaxon_pjrt.so"))
        except Exception as e:
            print(f"ntff hook install failed ({e}); timing disabled")


def kernel(x, W1, b1, W2, b2, src_ids, dst_ids, n_dst):
    n_dst = int(n_dst)
    assert n_dst == N_DST
    in_maps, tpb, assign = _host_prep(x, W1, b1, W2, b2, src_ids, dst_ids)
    if tpb not in _CACHE:
        _CACHE.clear()
        _CACHE[tpb] = _build(tpb)
    nc = _CACHE[tpb]
    trace = bool(os.environ.get("BASS_KERNEL_TRACE"))
    kw = {}
    if trace:
        _install_ntff_hook()
        kw = dict(trace=True, trace_cores=[0], stitch_traces=False)
    res = run_bass_kernel_spmd(nc, in_maps, core_ids=list(range(P)), **kw)
    if trace:
        print(f"HW exec time: {res.exec_time_ns} ns")
        if res.per_core_scope_times:
            for scope, m in sorted(res.per_core_scope_times.items()):
                print(f"  scope {scope}: {m}")
        if res.instructions_and_trace:
            print(f"  trace: {res.instructions_and_trace[1]}")
    out = np.zeros((NBLK * 128, OUTF), dtype=np.float32)
    for c in range(P):
        didx = (assign[c][:, None] * 128 + np.arange(128)[None, :]).reshape(-1)
        out[didx] = np.asarray(res.results[c]["out"], np.float32)
    return np.ascontiguousarray(out[:N_DST])


if __name__ == "__main__":
    # smoke test with random data
    rng = np.random.default_rng(0)
    x = rng.standard_normal((N_SRC, INF), dtype=np.float32)
    W1 = rng.standard_normal((OUTF, INF), dtype=np.float32) * 0.0625
    W2 = rng.standard_normal((OUTF, INF), dtype=np.float32) * 0.0625
    b1 = np.zeros(OUTF, np.float32)
    b2 = np.zeros(OUTF, np.float32)
    src = rng.integers(0, N_SRC, N_EDGES).astype(np.int32)
    dst = np.sort(rng.integers(0, N_DST, N_EDGES).astype(np.int32))
    got = kernel(x, W1, b1, W2, b2, src, dst, N_DST)
    proj = x @ W1.T + b1
    want = np.zeros((N_DST, OUTF), np.float32)
    np.add.at(want, dst, proj[src])
    want += x[:N_DST] @ W2.T + b2
    denom = np.abs(want).max()
    print("rel err:", np.abs(got - want).max() / denom)
